# revision 1
# baseline (speedup 1.0000x reference)
"""Trainium2 Bass kernel for nn_MultiHeadAttention_68865505624655.

Strategy (head parallelism, 8 cores x 2 heads):
  The reference's reshape(B,-1,T,H) mixes time/channel dims. For head h the
  per-head matrices are exactly reinterpretations of the compacted projection
  output Y_h = X @ W[h::16].T (shape (3072, 64)):
      Q_h^T (xi, t2)  == Y_h viewed as (64, 3072)   (same linear memory!)
      K_h^T (xi, t2)  == same
      V_h  (t2', xi)  == transpose of that view     (needs a real transpose)
  Each core:
    1. fused QKV projection for its 2 heads: Y6 = X @ [Wq1|Wk1|Wv1|Wq2|Wk2|Wv2]^T
       (24 t-blocks x 8 k-tiles of matmuls), + bias, written to DRAM scratch.
    2. reads back Q^T/K^T as (64,3072) contiguous views; V via PE transposes.
    3. energy^T tiles S^T[c,r] = K^T[:,c]^T.T @ Q^T (row-tiled pair, the 2
       heads run concurrently in disjoint PE row groups; float32r operands),
       exp on ScalarE in (128,1536) batches (no max subtraction -- energies
       are bounded ~|S|<60 so fp32 exp cannot overflow), written as bf16.
       Then one bf16 matmul per (c,head) with lhsT = [V_c | 1] (M=65)
       accumulates BOTH out^T[xi,r] (rows 0:64) and the softmax denominator
       Sigma[r] (row 64) over c in PSUM.
    4. writes per-head [out^T; Sigma] (65,3072) tiles per core.
  Host: divide rows 0:64 by row 64, interleave heads into (T,D), gamma*out+x.
  Toolchain workarounds: _split_multiwaits (this walrus allows one sync wait
  per instruction) and _install_ntff_shim (axon NTFF profiling hook).
"""

import sys

if "/opt/trn_rl_repo" not in sys.path:
    sys.path.insert(0, "/opt/trn_rl_repo")

import numpy as np


def _install_ntff_shim():
    """concourse.bass_utils under axon imports antenv.axon_hooks when
    tracing is requested; this image's antenv lacks that submodule.
    Register an equivalent shim (backed by the boot image's ctypes NTFF
    driver) so BASS_TRACE=1 profiles instead of crashing."""
    import types

    if "antenv.axon_hooks" in sys.modules:
        return
    mod = types.ModuleType("antenv.axon_hooks")
    cell = {}

    def get_axon_ntff_profile_hook():
        if "h" not in cell:
            try:
                from trn_agent_boot.trn_boot import _ntff_profile_via_ctypes
                cell["h"] = _ntff_profile_via_ctypes("/opt/axon/libaxon_pjrt.so")
            except Exception:
                cell["h"] = None
        return cell["h"]

    def set_axon_ntff_profile_hook(h):
        cell["h"] = h

    mod.get_axon_ntff_profile_hook = get_axon_ntff_profile_hook
    mod.set_axon_ntff_profile_hook = set_axon_ntff_profile_hook
    sys.modules["antenv.axon_hooks"] = mod


_install_ntff_shim()

import concourse.bass as bass
import concourse.mybir as mybir
import concourse.tile as tile
from concourse.bass import ds, ts
from concourse.masks import make_identity

F32 = mybir.dt.float32
F32R = mybir.dt.float32r
BF16 = mybir.dt.bfloat16

T = 3072          # sequence length (and t2 size)
D = 1024          # model dim
H = 16            # heads
NCORE = 8
EG = 64           # channel groups per head (columns of Y_h)
XI = 64           # "feature" dim of the quirky attention (t // 48)
NKT = D // 128    # 8 contraction tiles for the projection
NTB = T // 128    # 24 t-blocks / c-tiles
RCH = 512         # r-chunk (free dim of energy/AV matmuls)
NR = T // RCH     # 6 r-chunks
W6 = 6 * EG       # 384 fused projection output columns


def _split_multiwaits(nc):
    """This toolchain's walrus accepts at most ONE sync wait per
    instruction (setupSyncWait: 'Too many sync wait commands'), but Tile
    attaches several. Hoist all but the last wait of each instruction onto
    same-engine NoOps inserted right before it — semantically identical
    (sem-ge waits executed in sequence)."""
    n = 0
    for fn in nc.m.functions:
        for bb in fn.blocks:
            insts = list(bb.instructions)
            out = []
            changed = False
            for inst in insts:
                si = inst.sync_info
                if si is not None and len(si.on_wait) > 1:
                    waits = list(si.on_wait)
                    for w in waits[:-1]:
                        n += 1
                        out.append(mybir.InstNoOp(
                            name=f"I-splitwait-{n}",
                            ins=[], outs=[], engine=inst.engine,
                            sync_info=mybir.SyncInfo(on_wait=[w], on_update=[]),
                        ))
                    inst.sync_info = mybir.SyncInfo(
                        on_wait=[waits[-1]], on_update=list(si.on_update)
                    )
                    changed = True
                out.append(inst)
            if changed:
                bb.instructions = out
    return n


def build_program():
    nc = bass.Bass()

    xT = nc.dram_tensor("xT", [NTB, 128, NKT, 128], F32R, kind="ExternalInput")
    w6 = nc.dram_tensor("w6", [D, W6], F32R, kind="ExternalInput")
    b6 = nc.dram_tensor("b6", [128, W6], F32, kind="ExternalInput")
    y6qk = nc.dram_tensor("y6qk", [4, T, EG], F32R, kind="Internal")
    y6v = nc.dram_tensor("y6v", [2, T, EG], BF16, kind="Internal")
    outT = nc.dram_tensor("outT", [2, XI + 1, T], F32, kind="ExternalOutput")

    with tile.TileContext(nc) as tc:
        with tc.tile_pool(name="const", bufs=1) as constp:
            w6_sb = constp.tile([128, NKT, W6], F32R)
            w6v = w6[:, :].rearrange("(k p) n -> k p n", p=128)
            for k in range(NKT):
                nc.scalar.dma_start(out=w6_sb[:, k, :], in_=w6v[k, :, :])
            b6_sb = constp.tile([128, W6], F32)
            nc.scalar.dma_start(out=b6_sb, in_=b6[:, :])
            # identity blocks at partitions 0:64 and 64:128 so the two heads'
            # V^T transposes run row-paired in the PE array
            ident = constp.tile([128, 64], BF16)
            nc.gpsimd.memset(ident, 0.0)
            make_identity(nc, ident[0:64, :], nomemset=True)
            make_identity(nc, ident[64:128, :], nomemset=True)
            ones_f32 = constp.tile([128, 1], F32)
            nc.gpsimd.memset(ones_f32, 1.0)
            kt_sb = constp.tile([128, T], F32R)   # rows 0:64 h1 K^T, 64:128 h2
            vt_sb = constp.tile([128, T], BF16)   # rows 0:64 h1 V^T, 64:128 h2
            # V tiles augmented with a ones column: [:, c, 0:64] = V_h c-tile,
            # [:, c, 64] = 1.0 so one matmul computes out^T AND Sigma (row 64)
            v1_sb = constp.tile([128, NTB, XI + 1], BF16)
            v2_sb = constp.tile([128, NTB, XI + 1], BF16)
            for vsb in (v1_sb, v2_sb):
                for c in range(NTB):
                    nc.vector.tensor_copy(vsb[:, c, XI:XI + 1], ones_f32)

            # ---------------- projection: Y6 = X @ W6^T + b6 ----------------
            with tc.tile_pool(name="xt", bufs=4) as xtp, \
                 tc.tile_pool(name="psy", bufs=4, space="PSUM") as psyp, \
                 tc.tile_pool(name="ysb", bufs=4) as ysbp:
                for j in range(NTB):
                    xt = xtp.tile([128, NKT, 128], F32R)
                    nc.sync.dma_start(out=xt, in_=xT[j, :, :, :])
                    psy = psyp.tile([128, W6], F32)
                    for k in range(NKT):
                        nc.tensor.matmul(
                            psy, xt[:, k, :], w6_sb[:, k, :],
                            start=(k == 0), stop=(k == NKT - 1),
                        )
                    psyv = psy.rearrange("p (h z e) -> p h z e", h=2, z=3)
                    b6v = b6_sb.rearrange("p (h z e) -> p h z e", h=2, z=3)
                    ysbqk = ysbp.tile([128, 2, 2, EG], F32R, name="ysbqk")
                    nc.vector.tensor_add(ysbqk, psyv[:, :, 0:2, :],
                                         b6v[:, :, 0:2, :])
                    ysbv = ysbp.tile([128, 2, EG], BF16, name="ysbv")
                    nc.vector.tensor_add(ysbv, psyv[:, :, 2, :],
                                         b6v[:, :, 2, :])
                    nc.scalar.dma_start(
                        out=y6qk[:, ts(j, 128), :].rearrange("q t e -> t q e"),
                        in_=ysbqk,
                    )
                    nc.scalar.dma_start(
                        out=y6v[:, ts(j, 128), :].rearrange("q t e -> t q e"),
                        in_=ysbv,
                    )

            # ------- load K^T / V^T as contiguous (64, 3072) reinterpretations
            # split across the two HWDGE rings (SP + ACT) to run in parallel
            for eng, srcap, bufap in (
                    (nc.sync, y6v[0, :, :], vt_sb[0:64, :]),
                    (nc.sync, y6v[1, :, :], vt_sb[64:128, :]),
                    (nc.sync, y6qk[1, :, :], kt_sb[0:64, :]),
                    (nc.scalar, y6qk[3, :, :], kt_sb[64:128, :])):
                eng.dma_start(
                    out=bufap.rearrange("p (a e) -> p a e", a=48),
                    in_=srcap.rearrange("(xi a) e -> xi a e", xi=64),
                )

            # ------- V tiles: true transpose of V^T chunks via the PE -------
            with tc.tile_pool(name="vtps", bufs=4, space="PSUM") as vtpsp:
                for c in range(NTB):
                    for vsb, row0 in ((v1_sb, 0), (v2_sb, 64)):
                        vp = vtpsp.tile([128, XI], BF16)
                        nc.tensor.transpose(
                            vp, vt_sb[row0:row0 + 64, ts(c, 128)],
                            ident[row0:row0 + 64, :],
                        )
                        nc.vector.tensor_copy(vsb[:, c, 0:XI], vp)

            # --------------------------- attention --------------------------
            # (c-tile, head) pairs are enumerated as g = 2c + head and packed
            # three 512-wide energy segments per PSUM slot so each ScalarE
            # exp instruction covers (128, 1536) -- amortizes ACT overhead.
            SEG = 3
            with tc.tile_pool(name="qt", bufs=2) as qtp, \
                 tc.tile_pool(name="eps", bufs=2, space="PSUM") as epp, \
                 tc.tile_pool(name="ex", bufs=3) as expool, \
                 tc.tile_pool(name="outp", bufs=1, space="PSUM") as outpp, \
                 tc.tile_pool(name="osb", bufs=4) as osbp:
                for r in range(NR):
                    qt = qtp.tile([128, RCH], F32R)
                    for q, row0 in ((0, 0), (2, 64)):
                        nc.sync.dma_start(
                            out=qt[row0:row0 + 64, :].rearrange(
                                "p (a e) -> p a e", a=RCH // EG),
                            in_=y6qk[q, :, :].rearrange(
                                "(xi a) e -> xi a e", xi=64)[
                                :, ds(r * (RCH // EG), RCH // EG), :],
                        )
                    outp1 = outpp.tile([XI + 1, RCH], F32)
                    outp2 = outpp.tile([XI + 1, RCH], F32)
                    ep = None
                    pend = []
                    for g in range(2 * NTB):
                        c, hl = divmod(g, 2)
                        s = g % SEG
                        if s == 0:
                            ep = epp.tile([128, SEG * RCH], F32)
                        row0 = hl * 64
                        nc.tensor.matmul(
                            ep[:, ds(s * RCH, RCH)],
                            kt_sb[row0:row0 + 64, ts(c, 128)],
                            qt[row0:row0 + 64, :],
                            start=True, stop=True,
                        )
                        pend.append((hl, c, s))
                        if s == SEG - 1:
                            ex = expool.tile([128, SEG * RCH], BF16)
                            nc.scalar.activation(
                                ex, ep, mybir.ActivationFunctionType.Exp
                            )
                            for phl, pc, ps in pend:
                                outp = outp1 if phl == 0 else outp2
                                vsb = v1_sb if phl == 0 else v2_sb
                                nc.tensor.matmul(
                                    outp[:, :], vsb[:, pc, :],
                                    ex[:, ds(ps * RCH, RCH)],
                                    start=(pc == 0), stop=(pc == NTB - 1),
                                )
                            pend = []
                    for outp, hl in ((outp1, 0), (outp2, 1)):
                        osb = osbp.tile([XI + 1, RCH], F32)
                        nc.vector.tensor_copy(osb, outp)
                        nc.gpsimd.dma_start(
                            out=outT[hl, :, ts(r, RCH)], in_=osb
                        )
    return nc


def make_in_maps(x, Wq, bq, Wk, bk, Wv, bv):
    X = np.ascontiguousarray(np.asarray(x, dtype=np.float32).reshape(T, D))
    # (NTB, 128, NKT, 128): [j, p, k, t] = X[128j+t, 128k+p] -- every SBUF
    # partition reads one contiguous 4KB run per projection slab DMA
    xTm = np.ascontiguousarray(
        X.reshape(NTB, 128, NKT, 128).transpose(0, 3, 2, 1)
    )
    in_maps = []
    for c in range(NCORE):
        wcols, bcols = [], []
        for h in (2 * c, 2 * c + 1):
            for W, b in ((Wq, bq), (Wk, bk), (Wv, bv)):
                wcols.append(np.asarray(W, np.float32)[h::H, :].T)
                bcols.append(np.asarray(b, np.float32)[h::H])
        w6m = np.ascontiguousarray(np.concatenate(wcols, axis=1))
        b6m = np.ascontiguousarray(
            np.broadcast_to(np.concatenate(bcols), (128, W6))
        )
        in_maps.append({"xT": xTm, "w6": w6m, "b6": b6m})
    return X, in_maps


def assemble(X, results, gamma):
    O = np.empty((T, EG, H), dtype=np.float32)
    for c in range(NCORE):
        res = results[c]
        for hl in range(2):
            h = 2 * c + hl
            onn = res["outT"][hl][0:XI, :]                # (64, 3072)
            s = res["outT"][hl][XI, :]                    # (3072,)
            O[:, :, h] = (onn / s[None, :]).T
    out = O.reshape(T, D)
    g = np.float32(np.asarray(gamma))
    return (g * out + X).reshape(1, 1, T, D).astype(np.float32)


_PROGRAM = None
last_run_info = {}


def kernel(x, Wq, bq, Wk, bk, Wv, bv, gamma):
    global _PROGRAM
    from concourse import bass_utils

    X, in_maps = make_in_maps(x, Wq, bq, Wk, bk, Wv, bv)
    if _PROGRAM is None:
        _PROGRAM = build_program()
        # required for this toolchain's walrus (1 sync wait per instruction);
        # applied here so CoreSim (which predates these NoOps) can still run
        # the unsplit program from build_program()
        _split_multiwaits(_PROGRAM)
    res = bass_utils.run_bass_kernel_spmd(
        _PROGRAM, in_maps, core_ids=list(range(NCORE))
    )
    last_run_info["exec_time_ns"] = res.exec_time_ns
    last_run_info["trace"] = res.instructions_and_trace
    return assemble(X, res.results, gamma)



# revision 4
# speedup vs baseline: 1.1285x; 1.1285x over previous
"""Trainium2 Bass kernel for nn_MultiHeadAttention_68865505624655.

Strategy (head parallelism, 8 cores x 2 heads):
  The reference's reshape(B,-1,T,H) mixes time/channel dims. For head h the
  per-head matrices are exactly reinterpretations of the compacted projection
  output Y_h = X @ W[h::16].T (shape (3072, 64)):
      Q_h^T (xi, t2)  == Y_h viewed as (64, 3072)   (same linear memory!)
      K_h^T (xi, t2)  == same
      V_h  (t2', xi)  == transpose of that view     (needs a real transpose)
  v2 (this file): everything bf16 on the PE (fp32 matmuls run at half rate:
  fp32_mode=HIGH streams 2 cycles/row), exp split across TWO engines, DMAs
  moved off ScalarE so ACT does nothing but exp.
  Each core:
    1. fused QKV projection for its 2 heads: Y6 = X @ [Wq1|Wk1|Wv1|Wq2|Wk2|Wv2]^T
       in bf16 (24 t-blocks x 8 k-tiles), + bias on DVE, written bf16 to DRAM.
    2. reads back Q^T/K^T/V^T as (64,3072) contiguous views (6KB runs/partition).
    3. attention, r-chunk (512) outer, c-tile (128) inner, software-pipelined:
         - energy pair S^T[c,r] = K_h^T.T @ Q_h^T for both heads concurrently
           in disjoint PE row groups (bf16, N=512)
         - exp: head0 on ScalarE (exact LUT exp -> bf16), head1 on VectorE via
           the Schraudolph bit trick: bf16(exp(x)) ~= bitcast_i16(round(
           x*128/ln2 + 16248.3)) -- one tensor_scalar (mult,add) per tile.
           Softmax needs no max-subtraction: |S| < ~70 so fp32/bf16 exp
           cannot overflow, and S*A+B stays inside int16.
         - AV (lagged 2 iters): one bf16 matmul per (c,head) with
           lhsT = [V_c | 1] (M=65) accumulates out^T[xi,r] AND the softmax
           denominator Sigma[r] (row 64) over c in PSUM.
         - V^T -> V PE transposes are interleaved into the r=0 iteration
           (row-paired for the two heads) so there is no serial prologue.
    4. writes per-head [out^T; Sigma] (65,3072) tiles per core.
  Host: divide rows 0:64 by row 64, interleave heads into (T,D), gamma*out+x.
  Toolchain workarounds: _split_multiwaits (this walrus allows one sync wait
  per instruction) and _install_ntff_shim (axon NTFF profiling hook).
"""

import sys

if "/opt/trn_rl_repo" not in sys.path:
    sys.path.insert(0, "/opt/trn_rl_repo")

import numpy as np
import ml_dtypes


def _install_ntff_shim():
    """concourse.bass_utils under axon imports antenv.axon_hooks when
    tracing is requested; this image's antenv lacks that submodule.
    Register an equivalent shim (backed by the boot image's ctypes NTFF
    driver) so BASS_TRACE=1 profiles instead of crashing."""
    import types

    if "antenv.axon_hooks" in sys.modules:
        return
    mod = types.ModuleType("antenv.axon_hooks")
    cell = {}

    def get_axon_ntff_profile_hook():
        if "h" not in cell:
            try:
                from trn_agent_boot.trn_boot import _ntff_profile_via_ctypes
                cell["h"] = _ntff_profile_via_ctypes("/opt/axon/libaxon_pjrt.so")
            except Exception:
                cell["h"] = None
        return cell["h"]

    def set_axon_ntff_profile_hook(h):
        cell["h"] = h

    mod.get_axon_ntff_profile_hook = get_axon_ntff_profile_hook
    mod.set_axon_ntff_profile_hook = set_axon_ntff_profile_hook
    sys.modules["antenv.axon_hooks"] = mod


_install_ntff_shim()

import concourse.bass as bass
import concourse.mybir as mybir
import concourse.tile as tile
from concourse.bass import ds, ts
from concourse.masks import make_identity

F32 = mybir.dt.float32
BF16 = mybir.dt.bfloat16
I16 = mybir.dt.int16
BF16_NP = ml_dtypes.bfloat16

T = 3072          # sequence length (and t2 size)
D = 1024          # model dim
H = 16            # heads
NCORE = 8
EG = 64           # channel groups per head (columns of Y_h)
XI = 64           # "feature" dim of the quirky attention (t // 48)
NKT = D // 128    # 8 contraction tiles for the projection
NTB = T // 128    # 24 t-blocks / c-tiles
RCH = 512         # r-chunk (free dim of energy/AV matmuls)
NR = T // RCH     # 6 r-chunks
W6 = 6 * EG       # 384 fused projection output columns

# Schraudolph fast-exp constants for a bf16 result via int16 bit pattern:
# bf16_bits(exp(x)) ~= round(x * 2^7/ln2 + (127*2^7 - c)), c ~= 7.7
SCH_A = 128.0 / float(np.log(2.0))
SCH_B = 16248.3


def _split_multiwaits(nc):
    """This toolchain's walrus accepts at most ONE sync wait per
    instruction (setupSyncWait: 'Too many sync wait commands'), but Tile
    attaches several. Hoist all but the last wait of each instruction onto
    same-engine NoOps inserted right before it — semantically identical
    (sem-ge waits executed in sequence)."""
    n = 0
    for fn in nc.m.functions:
        for bb in fn.blocks:
            insts = list(bb.instructions)
            out = []
            changed = False
            for inst in insts:
                si = inst.sync_info
                if si is not None and len(si.on_wait) > 1:
                    waits = list(si.on_wait)
                    for w in waits[:-1]:
                        n += 1
                        out.append(mybir.InstNoOp(
                            name=f"I-splitwait-{n}",
                            ins=[], outs=[], engine=inst.engine,
                            sync_info=mybir.SyncInfo(on_wait=[w], on_update=[]),
                        ))
                    inst.sync_info = mybir.SyncInfo(
                        on_wait=[waits[-1]], on_update=list(si.on_update)
                    )
                    changed = True
                out.append(inst)
            if changed:
                bb.instructions = out
    return n


def build_program():
    nc = bass.Bass()

    xT = nc.dram_tensor("xT", [NTB, 128, NKT, 128], BF16, kind="ExternalInput")
    w6 = nc.dram_tensor("w6", [D, W6], BF16, kind="ExternalInput")
    b6 = nc.dram_tensor("b6", [128, W6], F32, kind="ExternalInput")
    y6 = nc.dram_tensor("y6", [6, T, EG], BF16, kind="Internal")
    outT = nc.dram_tensor("outT", [2, XI + 1, T], F32, kind="ExternalOutput")

    with tile.TileContext(nc) as tc:
        with tc.tile_pool(name="const", bufs=1) as constp:
            w6_sb = constp.tile([128, NKT, W6], BF16)
            w6v = w6[:, :].rearrange("(k p) n -> k p n", p=128)
            for k in range(NKT):
                nc.sync.dma_start(out=w6_sb[:, k, :], in_=w6v[k, :, :])
            b6_sb = constp.tile([128, W6], F32)
            nc.sync.dma_start(out=b6_sb, in_=b6[:, :])
            # identity blocks at partitions 0:64 and 64:128 so the two heads'
            # V^T transposes run row-paired in the PE array
            ident = constp.tile([128, 64], BF16)
            nc.gpsimd.memset(ident, 0.0)
            make_identity(nc, ident[0:64, :], nomemset=True)
            make_identity(nc, ident[64:128, :], nomemset=True)
            kt_sb = constp.tile([128, T], BF16)   # rows 0:64 h1 K^T, 64:128 h2
            vt_sb = constp.tile([128, T], BF16)   # rows 0:64 h1 V^T, 64:128 h2
            # V tiles augmented with a ones column: [:, c, 0:64] = V_h c-tile,
            # [:, c, 64] = 1.0 so one matmul computes out^T AND Sigma (row 64)
            v1_sb = constp.tile([128, NTB, XI + 1], BF16)
            v2_sb = constp.tile([128, NTB, XI + 1], BF16)
            nc.vector.memset(v1_sb[:, :, XI:XI + 1], 1.0)
            nc.vector.memset(v2_sb[:, :, XI:XI + 1], 1.0)

            # ---------------- projection: Y6 = X @ W6^T + b6 ----------------
            with tc.tile_pool(name="xt", bufs=4) as xtp, \
                 tc.tile_pool(name="psy", bufs=4, space="PSUM") as psyp, \
                 tc.tile_pool(name="ysb", bufs=4) as ysbp:
                for j in range(NTB):
                    xt = xtp.tile([128, NKT, 128], BF16)
                    nc.sync.dma_start(out=xt, in_=xT[j, :, :, :])
                    psy = psyp.tile([128, W6], F32)
                    for k in range(NKT):
                        nc.tensor.matmul(
                            psy, xt[:, k, :], w6_sb[:, k, :],
                            start=(k == 0), stop=(k == NKT - 1),
                        )
                    ysb = ysbp.tile([128, W6], BF16)
                    nc.vector.tensor_add(ysb, psy, b6_sb)
                    nc.gpsimd.dma_start(
                        out=y6[:, ts(j, 128), :].rearrange("q t e -> t q e"),
                        in_=ysb.rearrange("t (q e) -> t q e", q=6),
                    )

            # ------- load K^T / V^T as contiguous (64, 3072) reinterpretations
            # on two different queues so they run in parallel
            for eng, q, bufap in (
                    (nc.sync, 1, kt_sb[0:64, :]),
                    (nc.sync, 4, kt_sb[64:128, :]),
                    (nc.gpsimd, 2, vt_sb[0:64, :]),
                    (nc.gpsimd, 5, vt_sb[64:128, :])):
                eng.dma_start(
                    out=bufap.rearrange("p (a e) -> p a e", a=48),
                    in_=y6[q, :, :].rearrange("(xi a) e -> xi a e", xi=64),
                )

            # --------------------------- attention --------------------------
            with tc.tile_pool(name="qt", bufs=2) as qtp, \
                 tc.tile_pool(name="eps", bufs=2, space="PSUM") as epp, \
                 tc.tile_pool(name="ex", bufs=3) as expool, \
                 tc.tile_pool(name="vtps", bufs=2, space="PSUM") as vtpsp, \
                 tc.tile_pool(name="outp", bufs=1, space="PSUM") as outpp, \
                 tc.tile_pool(name="osb", bufs=4) as osbp:
                for r in range(NR):
                    qt = qtp.tile([128, RCH], BF16)
                    for q, row0 in ((0, 0), (3, 64)):
                        nc.sync.dma_start(
                            out=qt[row0:row0 + 64, :].rearrange(
                                "p (a e) -> p a e", a=RCH // EG),
                            in_=y6[q, :, :].rearrange(
                                "(xi a) e -> xi a e", xi=64)[
                                :, ds(r * (RCH // EG), RCH // EG), :],
                        )
                    outp1 = outpp.tile([XI + 1, RCH], F32)
                    outp2 = outpp.tile([XI + 1, RCH], F32)

                    def av(c):
                        for outp, vsb, ex_t in ((outp1, v1_sb, exq[c][0]),
                                                (outp2, v2_sb, exq[c][1])):
                            nc.tensor.matmul(
                                outp[:, :], vsb[:, c, :], ex_t,
                                start=(c == 0), stop=(c == NTB - 1),
                            )

                    exq = {}
                    for c in range(NTB):
                        ep = epp.tile([128, 2, RCH], F32)
                        for hl, row0 in ((0, 0), (1, 64)):
                            nc.tensor.matmul(
                                ep[:, hl, :],
                                kt_sb[row0:row0 + 64, ts(c, 128)],
                                qt[row0:row0 + 64, :],
                                start=True, stop=True,
                            )
                        ex_t = expool.tile([128, 2, RCH], BF16)
                        # head0: exact exp on ScalarE; head1: Schraudolph
                        # bit-trick exp on VectorE (int16 view of bf16 tile)
                        nc.scalar.activation(
                            ex_t[:, 0, :], ep[:, 0, :],
                            mybir.ActivationFunctionType.Exp,
                        )
                        nc.vector.tensor_scalar(
                            ex_t[:, 1, :].bitcast(I16), ep[:, 1, :],
                            SCH_A, SCH_B,
                            mybir.AluOpType.mult, mybir.AluOpType.add,
                        )
                        exq[c] = (ex_t[:, 0, :], ex_t[:, 1, :])
                        if r == 0:
                            # interleave the V^T -> V transposes (row-paired)
                            vp = vtpsp.tile([128, XI], BF16)
                            for vsb, row0 in ((v1_sb, 0), (v2_sb, 64)):
                                nc.tensor.transpose(
                                    vp, vt_sb[row0:row0 + 64, ts(c, 128)],
                                    ident[row0:row0 + 64, :],
                                )
                                nc.vector.tensor_copy(vsb[:, c, 0:XI], vp)
                        if c >= 2:
                            av(c - 2)
                    av(NTB - 2)
                    av(NTB - 1)
                    for outp, hl in ((outp1, 0), (outp2, 1)):
                        osb = osbp.tile([XI + 1, RCH], F32)
                        nc.vector.tensor_copy(osb, outp)
                        nc.gpsimd.dma_start(
                            out=outT[hl, :, ts(r, RCH)], in_=osb
                        )
    return nc


def make_in_maps(x, Wq, bq, Wk, bk, Wv, bv):
    X = np.ascontiguousarray(np.asarray(x, dtype=np.float32).reshape(T, D))
    # (NTB, 128, NKT, 128): [j, p, k, t] = X[128j+t, 128k+p] -- every SBUF
    # partition reads one contiguous run per projection slab DMA
    xTm = np.ascontiguousarray(
        X.astype(BF16_NP).reshape(NTB, 128, NKT, 128).transpose(0, 3, 2, 1)
    )
    in_maps = []
    for c in range(NCORE):
        wcols, bcols = [], []
        for h in (2 * c, 2 * c + 1):
            for W, b in ((Wq, bq), (Wk, bk), (Wv, bv)):
                wcols.append(np.asarray(W, np.float32)[h::H, :].T)
                bcols.append(np.asarray(b, np.float32)[h::H])
        w6m = np.ascontiguousarray(
            np.concatenate(wcols, axis=1).astype(BF16_NP)
        )
        b6m = np.ascontiguousarray(
            np.broadcast_to(np.concatenate(bcols), (128, W6)), dtype=np.float32
        )
        in_maps.append({"xT": xTm, "w6": w6m, "b6": b6m})
    return X, in_maps


def assemble(X, results, gamma):
    O = np.empty((T, EG, H), dtype=np.float32)
    for c in range(NCORE):
        res = results[c]
        for hl in range(2):
            h = 2 * c + hl
            onn = res["outT"][hl][0:XI, :]                # (64, 3072)
            s = res["outT"][hl][XI, :]                    # (3072,)
            O[:, :, h] = (onn / s[None, :]).T
    out = O.reshape(T, D)
    g = np.float32(np.asarray(gamma))
    return (g * out + X).reshape(1, 1, T, D).astype(np.float32)


_PROGRAM = None
last_run_info = {}


def kernel(x, Wq, bq, Wk, bk, Wv, bv, gamma):
    global _PROGRAM
    from concourse import bass_utils

    X, in_maps = make_in_maps(x, Wq, bq, Wk, bk, Wv, bv)
    if _PROGRAM is None:
        _PROGRAM = build_program()
        # required for this toolchain's walrus (1 sync wait per instruction);
        # applied here so CoreSim (which predates these NoOps) can still run
        # the unsplit program from build_program()
        _split_multiwaits(_PROGRAM)
    res = bass_utils.run_bass_kernel_spmd(
        _PROGRAM, in_maps, core_ids=list(range(NCORE))
    )
    last_run_info["exec_time_ns"] = res.exec_time_ns
    last_run_info["trace"] = res.instructions_and_trace
    return assemble(X, res.results, gamma)


# revision 13
# speedup vs baseline: 1.1819x; 1.0474x over previous
"""Trainium2 Bass kernel for nn_MultiHeadAttention_68865505624655.

Strategy (head parallelism, 8 cores x 2 heads):
  The reference's reshape(B,-1,T,H) mixes time/channel dims. For head h the
  per-head matrices are exactly reinterpretations of the compacted projection
  output Y_h = X @ W[h::16].T (shape (3072, 64)):
      Q_h^T (xi, t2)  == Y_h viewed as (64, 3072)   (same linear memory!)
      K_h^T (xi, t2)  == same
      V_h  (t2', xi)  == transpose of that view     (needs a real transpose)
  v3 (this file): everything bf16 on the PE (fp32 matmuls run at half rate:
  fp32_mode=HIGH streams 2 cycles/row), exp alternates between TWO engines
  in wide FD=1024 instructions so the PE never stalls on softmax, and every
  DMA stream is spread across otherwise-idle engine queues.
  Each core:
    1. fused QKV projection for its 2 heads: Y6 = X @ [Wq1|Wk1|Wv1|Wq2|Wk2|Wv2]^T
       in bf16 (24 t-blocks x 8 k-tiles), + bias on DVE, written bf16 to DRAM.
    2. reads back Q^T/K^T/V^T as (64,3072) contiguous views (6KB runs/partition),
       four loads on four queues; V^T -> V PE transposes (row-paired) fill the
       PE-idle bubble while those loads stream.
    3. attention, r-chunk (512) outer, c-tile (128) inner, software-pipelined:
         - energy pair S^T[c,r] = K_h^T.T @ Q_h^T for both heads concurrently
           in disjoint PE row groups (bf16, N=512)
         - exp over both heads in ONE FD=1024 instruction, alternating by c
           parity: even c on VectorE via the Schraudolph bit trick
           bf16(exp(x)) ~= bitcast_i16(round(x*128/ln2 + 16248.3)) (a single
           tensor_scalar mult+add); odd c on ScalarE (exact LUT exp).
           Softmax needs no max-subtraction: |S| < ~70 so fp32/bf16 exp
           cannot overflow, and S*A+B stays inside int16.
         - AV (lagged 2 iters): one bf16 matmul per (c,head) with
           lhsT = [V_c | 1] (M=65) accumulates out^T[xi,r] AND the softmax
           denominator Sigma[r] (row 64) over c in PSUM.
    4. writes per-head [out^T; Sigma] (65,3072) tiles per core.
  Host: divide rows 0:64 by row 64, interleave heads into (T,D), gamma*out+x.
  Toolchain workarounds: _split_multiwaits (this walrus allows one sync wait
  per instruction) and _install_ntff_shim (axon NTFF profiling hook).
"""

import sys

if "/opt/trn_rl_repo" not in sys.path:
    sys.path.insert(0, "/opt/trn_rl_repo")

import numpy as np
import ml_dtypes


def _install_ntff_shim():
    """concourse.bass_utils under axon imports antenv.axon_hooks when
    tracing is requested; this image's antenv lacks that submodule.
    Register an equivalent shim (backed by the boot image's ctypes NTFF
    driver) so BASS_TRACE=1 profiles instead of crashing."""
    import types

    if "antenv.axon_hooks" in sys.modules:
        return
    mod = types.ModuleType("antenv.axon_hooks")
    cell = {}

    def get_axon_ntff_profile_hook():
        if "h" not in cell:
            try:
                from trn_agent_boot.trn_boot import _ntff_profile_via_ctypes
                cell["h"] = _ntff_profile_via_ctypes("/opt/axon/libaxon_pjrt.so")
            except Exception:
                cell["h"] = None
        return cell["h"]

    def set_axon_ntff_profile_hook(h):
        cell["h"] = h

    mod.get_axon_ntff_profile_hook = get_axon_ntff_profile_hook
    mod.set_axon_ntff_profile_hook = set_axon_ntff_profile_hook
    sys.modules["antenv.axon_hooks"] = mod


_install_ntff_shim()

import concourse.bass as bass
import concourse.mybir as mybir
import concourse.tile as tile
from concourse.bass import ds, ts
from concourse.masks import make_identity

F32 = mybir.dt.float32
BF16 = mybir.dt.bfloat16
I16 = mybir.dt.int16
BF16_NP = ml_dtypes.bfloat16

T = 3072          # sequence length (and t2 size)
D = 1024          # model dim
H = 16            # heads
NCORE = 8
EG = 64           # channel groups per head (columns of Y_h)
XI = 64           # "feature" dim of the quirky attention (t // 48)
NKT = D // 128    # 8 contraction tiles for the projection
NTB = T // 128    # 24 t-blocks / c-tiles
RCH = 512         # r-chunk (free dim of energy/AV matmuls)
NR = T // RCH     # 6 r-chunks
W6 = 6 * EG       # 384 fused projection output columns

# Schraudolph fast-exp constants for a bf16 result via int16 bit pattern:
# bf16_bits(exp(x)) ~= round(x * 2^7/ln2 + (127*2^7 - c)), c ~= 7.7
SCH_A = 128.0 / float(np.log(2.0))
SCH_B = 16248.3


def _split_multiwaits(nc):
    """This toolchain's walrus accepts at most ONE sync wait per
    instruction (setupSyncWait: 'Too many sync wait commands'), but Tile
    attaches several. Hoist all but the last wait of each instruction onto
    same-engine NoOps inserted right before it — semantically identical
    (sem-ge waits executed in sequence)."""
    n = 0
    for fn in nc.m.functions:
        for bb in fn.blocks:
            insts = list(bb.instructions)
            out = []
            changed = False
            for inst in insts:
                si = inst.sync_info
                if si is not None and len(si.on_wait) > 1:
                    waits = list(si.on_wait)
                    for w in waits[:-1]:
                        n += 1
                        out.append(mybir.InstNoOp(
                            name=f"I-splitwait-{n}",
                            ins=[], outs=[], engine=inst.engine,
                            sync_info=mybir.SyncInfo(on_wait=[w], on_update=[]),
                        ))
                    inst.sync_info = mybir.SyncInfo(
                        on_wait=[waits[-1]], on_update=list(si.on_update)
                    )
                    changed = True
                out.append(inst)
            if changed:
                bb.instructions = out
    return n


def build_program():
    nc = bass.Bass()

    xT = nc.dram_tensor("xT", [NTB, 128, NKT, 128], BF16, kind="ExternalInput")
    w6 = nc.dram_tensor("w6", [D, W6], BF16, kind="ExternalInput")
    b6 = nc.dram_tensor("b6", [128, W6], F32, kind="ExternalInput")
    y6 = nc.dram_tensor("y6", [6, T, EG], BF16, kind="Internal")
    outT = nc.dram_tensor("outT", [2, XI + 1, T], F32, kind="ExternalOutput")

    with tile.TileContext(nc) as tc:
        with tc.tile_pool(name="const", bufs=1) as constp:
            w6_sb = constp.tile([128, NKT, W6], BF16)
            w6v = w6[:, :].rearrange("(k p) n -> k p n", p=128)
            # spread the startup loads across all three DMA-capable queues
            # so the first projection matmul isn't gated on one serial stream
            w6_engs = (nc.sync, nc.scalar, nc.gpsimd)
            for k in range(NKT):
                w6_engs[k % 3].dma_start(out=w6_sb[:, k, :], in_=w6v[k, :, :])
            b6_sb = constp.tile([128, W6], F32)
            nc.scalar.dma_start(out=b6_sb, in_=b6[:, :])
            # identity blocks at partitions 0:64 and 64:128 so the two heads'
            # V^T transposes run row-paired in the PE array
            ident = constp.tile([128, 64], BF16)
            nc.gpsimd.memset(ident, 0.0)
            make_identity(nc, ident[0:64, :], nomemset=True)
            make_identity(nc, ident[64:128, :], nomemset=True)
            kt_sb = constp.tile([128, T], BF16)   # rows 0:64 h1 K^T, 64:128 h2
            vt_sb = constp.tile([128, T], BF16)   # rows 0:64 h1 V^T, 64:128 h2
            # V tiles augmented with a ones column: [:, c, 0:64] = V_h c-tile,
            # [:, c, 64] = 1.0 so one matmul computes out^T AND Sigma (row 64)
            v1_sb = constp.tile([128, NTB, XI + 1], BF16)
            v2_sb = constp.tile([128, NTB, XI + 1], BF16)
            nc.vector.memset(v1_sb[:, :, XI:XI + 1], 1.0)
            nc.vector.memset(v2_sb[:, :, XI:XI + 1], 1.0)

            # ---------------- projection: Y6 = X @ W6^T + b6 ----------------
            with tc.tile_pool(name="xt", bufs=4) as xtp, \
                 tc.tile_pool(name="psy", bufs=4, space="PSUM") as psyp, \
                 tc.tile_pool(name="ysb", bufs=4) as ysbp:
                for j in range(NTB):
                    xt = xtp.tile([128, NKT, 128], BF16)
                    # alternate queues: one queue can't sustain the 200+ GB/s
                    # this stream needs to stay hidden under the matmuls
                    (nc.sync if j % 2 == 0 else nc.gpsimd).dma_start(
                        out=xt, in_=xT[j, :, :, :])
                    psy = psyp.tile([128, W6], F32)
                    for k in range(NKT):
                        nc.tensor.matmul(
                            psy, xt[:, k, :], w6_sb[:, k, :],
                            start=(k == 0), stop=(k == NKT - 1),
                        )
                    ysb = ysbp.tile([128, W6], BF16)
                    nc.vector.tensor_add(ysb, psy, b6_sb)
                    nc.scalar.dma_start(
                        out=y6[:, ts(j, 128), :].rearrange("q t e -> t q e"),
                        in_=ysb.rearrange("t (q e) -> t q e", q=6),
                    )

            # ------- load K^T / V^T as contiguous (64, 3072) reinterpretations
            # spread over the three DMA queues; V^T first (the PE transposes
            # consume it first), K^T halves on two different queues
            for eng, q, bufap in (
                    (nc.gpsimd, 2, vt_sb[0:64, :]),
                    (nc.scalar, 5, vt_sb[64:128, :]),
                    (nc.sync, 1, kt_sb[0:64, :]),
                    (nc.gpsimd, 4, kt_sb[64:128, :])):
                eng.dma_start(
                    out=bufap.rearrange("p (a e) -> p a e", a=48),
                    in_=y6[q, :, :].rearrange("(xi a) e -> xi a e", xi=64),
                )

            # ------- V tiles: true transpose of V^T chunks via the PE, filling
            # the PE-idle bubble while K^T/Q^T stream in
            with tc.tile_pool(name="vtps", bufs=2, space="PSUM") as vtpsp:
                for c in range(NTB):
                    for vsb, row0 in ((v1_sb, 0), (v2_sb, 64)):
                        vp = vtpsp.tile([128, XI], BF16)
                        nc.tensor.transpose(
                            vp, vt_sb[row0:row0 + 64, ts(c, 128)],
                            ident[row0:row0 + 64, :],
                        )
                        nc.vector.tensor_copy(vsb[:, c, 0:XI], vp)

            # --------------------------- attention --------------------------
            with tc.tile_pool(name="qt", bufs=2) as qtp, \
                 tc.tile_pool(name="eps", bufs=2, space="PSUM") as epp, \
                 tc.tile_pool(name="ex", bufs=3) as expool, \
                 tc.tile_pool(name="outp", bufs=1, space="PSUM") as outpp, \
                 tc.tile_pool(name="osb", bufs=4) as osbp:
                for r in range(NR):
                    qt = qtp.tile([128, RCH], BF16)
                    for q, row0 in ((0, 0), (3, 64)):
                        # r=0 on the scalar queue (idle until the first odd-c
                        # exp), in parallel with the kt/vt loads elsewhere
                        eng = nc.scalar if r == 0 else nc.sync
                        eng.dma_start(
                            out=qt[row0:row0 + 64, :].rearrange(
                                "p (a e) -> p a e", a=RCH // EG),
                            in_=y6[q, :, :].rearrange(
                                "(xi a) e -> xi a e", xi=64)[
                                :, ds(r * (RCH // EG), RCH // EG), :],
                        )
                    outp1 = outpp.tile([XI + 1, RCH], F32)
                    outp2 = outpp.tile([XI + 1, RCH], F32)

                    def av(c):
                        for outp, vsb, ex_t in ((outp1, v1_sb, exq[c][0]),
                                                (outp2, v2_sb, exq[c][1])):
                            nc.tensor.matmul(
                                outp[:, :], vsb[:, c, :], ex_t,
                                start=(c == 0), stop=(c == NTB - 1),
                            )

                    exq = {}
                    for c in range(NTB):
                        ep = epp.tile([128, 2, RCH], F32)
                        for hl, row0 in ((0, 0), (1, 64)):
                            nc.tensor.matmul(
                                ep[:, hl, :],
                                kt_sb[row0:row0 + 64, ts(c, 128)],
                                qt[row0:row0 + 64, :],
                                start=True, stop=True,
                            )
                        ex_t = expool.tile([128, 2, RCH], BF16)
                        # both heads' exp in ONE wide (FD=1024) instruction,
                        # alternating engines by c parity: even c -> VectorE
                        # Schraudolph bit-trick (int16 view of the bf16 tile),
                        # odd c -> ScalarE exact LUT exp. Each engine then
                        # runs one ~1.05-1.1us instruction every other
                        # iteration, comfortably under the PE's ~1.3us pair,
                        # so the PE stream stays dense and HAM stays warm.
                        if c % 2 == 0:
                            nc.vector.tensor_scalar(
                                ex_t[:, :, :].bitcast(I16), ep[:, :, :],
                                SCH_A, SCH_B,
                                mybir.AluOpType.mult, mybir.AluOpType.add,
                            )
                        else:
                            nc.scalar.activation(
                                ex_t[:, :, :], ep[:, :, :],
                                mybir.ActivationFunctionType.Exp,
                            )
                        exq[c] = (ex_t[:, 0, :], ex_t[:, 1, :])
                        if c >= 2:
                            av(c - 2)
                    av(NTB - 2)
                    av(NTB - 1)
                    for outp, hl in ((outp1, 0), (outp2, 1)):
                        osb = osbp.tile([XI + 1, RCH], F32)
                        nc.vector.tensor_copy(osb, outp)
                        (nc.scalar if hl == 0 else nc.gpsimd).dma_start(
                            out=outT[hl, :, ts(r, RCH)], in_=osb
                        )
    return nc


def make_in_maps(x, Wq, bq, Wk, bk, Wv, bv):
    X = np.ascontiguousarray(np.asarray(x, dtype=np.float32).reshape(T, D))
    # (NTB, 128, NKT, 128): [j, p, k, t] = X[128j+t, 128k+p] -- every SBUF
    # partition reads one contiguous run per projection slab DMA
    xTm = np.ascontiguousarray(
        X.astype(BF16_NP).reshape(NTB, 128, NKT, 128).transpose(0, 3, 2, 1)
    )
    in_maps = []
    for c in range(NCORE):
        wcols, bcols = [], []
        for h in (2 * c, 2 * c + 1):
            for W, b in ((Wq, bq), (Wk, bk), (Wv, bv)):
                wcols.append(np.asarray(W, np.float32)[h::H, :].T)
                bcols.append(np.asarray(b, np.float32)[h::H])
        w6m = np.ascontiguousarray(
            np.concatenate(wcols, axis=1).astype(BF16_NP)
        )
        b6m = np.ascontiguousarray(
            np.broadcast_to(np.concatenate(bcols), (128, W6)), dtype=np.float32
        )
        in_maps.append({"xT": xTm, "w6": w6m, "b6": b6m})
    return X, in_maps


def assemble(X, results, gamma):
    O = np.empty((T, EG, H), dtype=np.float32)
    for c in range(NCORE):
        res = results[c]
        for hl in range(2):
            h = 2 * c + hl
            onn = res["outT"][hl][0:XI, :]                # (64, 3072)
            s = res["outT"][hl][XI, :]                    # (3072,)
            O[:, :, h] = (onn / s[None, :]).T
    out = O.reshape(T, D)
    g = np.float32(np.asarray(gamma))
    return (g * out + X).reshape(1, 1, T, D).astype(np.float32)


_PROGRAM = None
last_run_info = {}


def kernel(x, Wq, bq, Wk, bk, Wv, bv, gamma):
    global _PROGRAM
    from concourse import bass_utils

    X, in_maps = make_in_maps(x, Wq, bq, Wk, bk, Wv, bv)
    if _PROGRAM is None:
        _PROGRAM = build_program()
        # required for this toolchain's walrus (1 sync wait per instruction);
        # applied here so CoreSim (which predates these NoOps) can still run
        # the unsplit program from build_program()
        _split_multiwaits(_PROGRAM)
    res = bass_utils.run_bass_kernel_spmd(
        _PROGRAM, in_maps, core_ids=list(range(NCORE))
    )
    last_run_info["exec_time_ns"] = res.exec_time_ns
    last_run_info["trace"] = res.instructions_and_trace
    return assemble(X, res.results, gamma)


# revision 20
# speedup vs baseline: 1.2242x; 1.0358x over previous
"""Trainium2 Bass kernel for nn_MultiHeadAttention_68865505624655.

Strategy (head parallelism, 8 cores x 2 heads):
  The reference's reshape(B,-1,T,H) mixes time/channel dims. For head h the
  per-head matrices are exactly reinterpretations of the compacted projection
  output Y_h = X @ W[h::16].T (shape (3072, 64)):
      Q_h^T (xi, t2)  == Y_h viewed as (64, 3072)   (same linear memory!)
      K_h^T (xi, t2)  == same
      V_h  (t2', xi)  == transpose of that view     (needs a real transpose)
  v3 (this file): everything bf16 on the PE (fp32 matmuls run at half rate:
  fp32_mode=HIGH streams 2 cycles/row), exp alternates between TWO engines
  in wide FD=1024 instructions so the PE never stalls on softmax, and every
  DMA stream is spread across otherwise-idle engine queues.
  Each core:
    1. fused QKV projection for its 2 heads: Y6 = X @ [Wq1|Wk1|Wv1|Wq2|Wk2|Wv2]^T
       in bf16 (24 t-blocks x 8 k-tiles), + bias on DVE, written bf16 to DRAM.
    2. reads back Q^T/K^T/V^T as (64,3072) contiguous views (6KB runs/partition),
       four loads on four queues; V^T -> V PE transposes (row-paired) fill the
       PE-idle bubble while those loads stream.
    3. attention, r-chunk (512) outer, c-tile (128) inner, software-pipelined:
         - energy pair S^T[c,r] = K_h^T.T @ Q_h^T for both heads concurrently
           in disjoint PE row groups (bf16, N=512)
         - exp over both heads in ONE FD=1024 instruction, alternating by c
           parity: even c on VectorE via the Schraudolph bit trick
           bf16(exp(x)) ~= bitcast_i16(round(x*128/ln2 + 16248.3)) (a single
           tensor_scalar mult+add); odd c on ScalarE (exact LUT exp).
           Softmax needs no max-subtraction: |S| < ~70 so fp32/bf16 exp
           cannot overflow, and S*A+B stays inside int16.
         - AV (lagged 2 iters): one bf16 matmul per (c,head) with
           lhsT = [V_c | 1] (M=65) accumulates out^T[xi,r] AND the softmax
           denominator Sigma[r] (row 64) over c in PSUM.
    4. writes per-head [out^T; Sigma] (65,3072) tiles per core.
  Host: divide rows 0:64 by row 64, interleave heads into (T,D), gamma*out+x.
  Toolchain workarounds: _split_multiwaits (this walrus allows one sync wait
  per instruction) and _install_ntff_shim (axon NTFF profiling hook).
"""

import sys

if "/opt/trn_rl_repo" not in sys.path:
    sys.path.insert(0, "/opt/trn_rl_repo")

import numpy as np
import ml_dtypes


def _install_ntff_shim():
    """concourse.bass_utils under axon imports antenv.axon_hooks when
    tracing is requested; this image's antenv lacks that submodule.
    Register an equivalent shim (backed by the boot image's ctypes NTFF
    driver) so BASS_TRACE=1 profiles instead of crashing."""
    import types

    if "antenv.axon_hooks" in sys.modules:
        return
    mod = types.ModuleType("antenv.axon_hooks")
    cell = {}

    def get_axon_ntff_profile_hook():
        if "h" not in cell:
            try:
                from trn_agent_boot.trn_boot import _ntff_profile_via_ctypes
                cell["h"] = _ntff_profile_via_ctypes("/opt/axon/libaxon_pjrt.so")
            except Exception:
                cell["h"] = None
        return cell["h"]

    def set_axon_ntff_profile_hook(h):
        cell["h"] = h

    mod.get_axon_ntff_profile_hook = get_axon_ntff_profile_hook
    mod.set_axon_ntff_profile_hook = set_axon_ntff_profile_hook
    sys.modules["antenv.axon_hooks"] = mod


_install_ntff_shim()

import concourse.bass as bass
import concourse.mybir as mybir
import concourse.tile as tile
from concourse.bass import ds, ts
from concourse.masks import make_identity

F32 = mybir.dt.float32
BF16 = mybir.dt.bfloat16
I16 = mybir.dt.int16
BF16_NP = ml_dtypes.bfloat16

T = 3072          # sequence length (and t2 size)
D = 1024          # model dim
H = 16            # heads
NCORE = 8
EG = 64           # channel groups per head (columns of Y_h)
XI = 64           # "feature" dim of the quirky attention (t // 48)
NKT = D // 128    # 8 contraction tiles for the projection
NTB = T // 128    # 24 t-blocks / c-tiles
RCH = 512         # r-chunk (free dim of energy/AV matmuls)
NR = T // RCH     # 6 r-chunks
W6 = 6 * EG       # 384 fused projection output columns

# Schraudolph fast-exp constants for a bf16 result via int16 bit pattern:
# bf16_bits(exp(x)) ~= round(x * 2^7/ln2 + (127*2^7 - c)), c ~= 7.7
SCH_A = 128.0 / float(np.log(2.0))
SCH_B = 16248.3


def _split_multiwaits(nc):
    """This toolchain's walrus accepts at most ONE sync wait per
    instruction (setupSyncWait: 'Too many sync wait commands'), but Tile
    attaches several. Hoist all but the last wait of each instruction onto
    same-engine NoOps inserted right before it — semantically identical
    (sem-ge waits executed in sequence)."""
    n = 0
    for fn in nc.m.functions:
        for bb in fn.blocks:
            insts = list(bb.instructions)
            out = []
            changed = False
            for inst in insts:
                si = inst.sync_info
                if si is not None and len(si.on_wait) > 1:
                    waits = list(si.on_wait)
                    for w in waits[:-1]:
                        n += 1
                        out.append(mybir.InstNoOp(
                            name=f"I-splitwait-{n}",
                            ins=[], outs=[], engine=inst.engine,
                            sync_info=mybir.SyncInfo(on_wait=[w], on_update=[]),
                        ))
                    inst.sync_info = mybir.SyncInfo(
                        on_wait=[waits[-1]], on_update=list(si.on_update)
                    )
                    changed = True
                out.append(inst)
            if changed:
                bb.instructions = out
    return n


def build_program():
    nc = bass.Bass()

    xT = nc.dram_tensor("xT", [NTB, 128, NKT, 128], BF16, kind="ExternalInput")
    w6 = nc.dram_tensor("w6", [D, W6], BF16, kind="ExternalInput")
    b6 = nc.dram_tensor("b6", [128, W6], F32, kind="ExternalInput")
    # y6 is declared in the READ layout: plane q viewed as (xi, t2) -- the
    # (64, 3072) reinterpretation of Y_h -- so the attention-side loads are
    # plain 2D slices with 6KB contiguous runs per partition (one DMA
    # descriptor burst each). The write side pays the scatter instead, where
    # consecutive-partition 128B chunks aggregate into large DRAM bursts.
    y6 = nc.dram_tensor("y6", [6, XI, T], BF16, kind="Internal")
    outT = nc.dram_tensor("outT", [2, XI + 1, T], F32, kind="ExternalOutput")

    with tile.TileContext(nc) as tc:
        with tc.tile_pool(name="const", bufs=1) as constp:
            w6_sb = constp.tile([128, NKT, W6], BF16)
            w6v = w6[:, :].rearrange("(k p) n -> k p n", p=128)
            # startup loads: keep nc.sync free for the first x-tile (critical
            # path to the first matmul); weights go on scalar/gpsimd
            w6_engs = (nc.scalar, nc.gpsimd)
            for k in range(NKT):
                w6_engs[k % 2].dma_start(out=w6_sb[:, k, :], in_=w6v[k, :, :])
            b6_sb = constp.tile([128, W6], F32)
            nc.scalar.dma_start(out=b6_sb, in_=b6[:, :])
            # identity blocks at partitions 0:64 and 64:128 so the two heads'
            # V^T transposes run row-paired in the PE array
            ident = constp.tile([128, 64], BF16)
            nc.gpsimd.memset(ident, 0.0)
            make_identity(nc, ident[0:64, :], nomemset=True)
            make_identity(nc, ident[64:128, :], nomemset=True)
            kt_sb = constp.tile([128, T], BF16)   # rows 0:64 h1 K^T, 64:128 h2
            vt_sb = constp.tile([128, T], BF16)   # rows 0:64 h1 V^T, 64:128 h2
            # V tiles augmented with a ones column: [:, c, h, 0:64] = V_h
            # c-tile, [:, c, h, 64] = 1.0 so one matmul computes out^T AND
            # Sigma (row 64)
            v12_sb = constp.tile([128, NTB, 2, XI + 1], BF16)
            nc.vector.memset(v12_sb[:, :, :, XI:XI + 1], 1.0)

            # ---------------- projection: Y6 = X @ W6^T + b6 ----------------
            with tc.tile_pool(name="xt", bufs=4) as xtp, \
                 tc.tile_pool(name="psy", bufs=4, space="PSUM") as psyp, \
                 tc.tile_pool(name="ysb", bufs=4) as ysbp:
                # y6 write view: plane q, row t = 48*xi + a, col e --
                # exactly the transpose scatter of a 128-row t-block
                y6w = y6.rearrange("q xi (a e) -> (xi a) q e", e=EG)
                for j in range(NTB):
                    xt = xtp.tile([128, NKT, 128], BF16)
                    # merged 1D free dim (2KB/partition contiguous both
                    # sides) and alternating queues: one queue can't sustain
                    # the rate this stream needs to hide under the matmuls
                    (nc.sync if j % 2 == 0 else nc.gpsimd).dma_start(
                        out=xt.rearrange("p k t -> p (k t)"),
                        in_=xT[j, :, :, :].rearrange("p k t -> p (k t)"))
                    psy = psyp.tile([128, W6], F32)
                    for k in range(NKT):
                        nc.tensor.matmul(
                            psy, xt[:, k, :], w6_sb[:, k, :],
                            start=(k == 0), stop=(k == NKT - 1),
                        )
                    ysb = ysbp.tile([128, W6], BF16)
                    nc.vector.tensor_add(ysb, psy, b6_sb)
                    nc.scalar.dma_start(
                        out=y6w[ts(j, 128), :, :],
                        in_=ysb.rearrange("t (q e) -> t q e", q=6),
                    )

            # ------- load K^T / V^T as contiguous (64, 3072) reinterpretations
            # (plain 2D slices of y6 -> 6KB descriptors); V^T first (the PE
            # transposes consume it first), spread over the three DMA queues
            for eng, q, bufap in (
                    (nc.gpsimd, 2, vt_sb[0:64, :]),
                    (nc.scalar, 5, vt_sb[64:128, :]),
                    (nc.sync, 1, kt_sb[0:64, :]),
                    (nc.gpsimd, 4, kt_sb[64:128, :])):
                eng.dma_start(out=bufap, in_=y6[q, :, :])

            # ------- V tiles: true transpose of V^T chunks via the PE, filling
            # the PE-idle bubble while K^T/Q^T stream in. Both heads write one
            # psum tile (disjoint row groups -> concurrent) drained by a
            # single DVE copy, so the scheduler can't unpair them.
            with tc.tile_pool(name="vtps", bufs=2, space="PSUM") as vtpsp:
                for c in range(NTB):
                    # [128, 2, 1024]: each head's transpose drains into its
                    # OWN psum bank (concurrent row-group matmuls must not
                    # share a bank), one strided DVE copy drains both
                    vp = vtpsp.tile([128, 2, 1024], BF16)
                    for hl, row0 in ((0, 0), (1, 64)):
                        nc.tensor.transpose(
                            vp[:, hl, 0:XI], vt_sb[row0:row0 + 64, ts(c, 128)],
                            ident[row0:row0 + 64, :],
                        )
                    nc.vector.tensor_copy(v12_sb[:, c, :, 0:XI], vp[:, :, 0:XI])

            # --------------------------- attention --------------------------
            # Pipeline depths: the exp of a (c) tile has ~1.7us latency from
            # its energy pair, so the energy matmul of (c+3) -- which reuses
            # the psum slot (eps bufs=3) -- and the AV matmuls of (c) -- which
            # consume the exp output (lag 3) -- both clear it with slack, and
            # the PE never waits.
            AVLAG = 3
            with tc.tile_pool(name="qt", bufs=2) as qtp, \
                 tc.tile_pool(name="eps", bufs=3, space="PSUM") as epp, \
                 tc.tile_pool(name="ex", bufs=5) as expool, \
                 tc.tile_pool(name="outp", bufs=1, space="PSUM") as outpp, \
                 tc.tile_pool(name="osb", bufs=4) as osbp:
                for r in range(NR):
                    qt = qtp.tile([128, RCH], BF16)
                    for q, row0 in ((0, 0), (3, 64)):
                        # r=0 on the scalar queue (idle until the first odd-c
                        # exp), in parallel with the kt/vt loads elsewhere
                        eng = nc.scalar if r == 0 else nc.sync
                        eng.dma_start(
                            out=qt[row0:row0 + 64, :],
                            in_=y6[q, :, ds(r * RCH, RCH)],
                        )
                    outp1 = outpp.tile([XI + 1, RCH], F32)
                    outp2 = outpp.tile([XI + 1, RCH], F32)

                    def av(c):
                        for hl, outp in ((0, outp1), (1, outp2)):
                            nc.tensor.matmul(
                                outp[:, :], v12_sb[:, c, hl, :], exq[c][hl],
                                start=(c == 0), stop=(c == NTB - 1),
                            )

                    exq = {}
                    for c in range(NTB):
                        ep = epp.tile([128, 2, RCH], F32)
                        for hl, row0 in ((0, 0), (1, 64)):
                            nc.tensor.matmul(
                                ep[:, hl, :],
                                kt_sb[row0:row0 + 64, ts(c, 128)],
                                qt[row0:row0 + 64, :],
                                start=True, stop=True,
                            )
                        ex_t = expool.tile([128, 2, RCH], BF16)
                        # both heads' exp in ONE wide (FD=1024) instruction,
                        # alternating engines by c parity: even c -> VectorE
                        # Schraudolph bit-trick (int16 view of the bf16 tile),
                        # odd c -> ScalarE exact LUT exp. Each engine then
                        # runs one ~1.1-1.2us instruction every other
                        # iteration, under the PE's ~1.4us pair of iterations,
                        # so the PE stream stays dense and HAM stays warm.
                        if c % 2 == 0:
                            nc.vector.tensor_scalar(
                                ex_t[:, :, :].bitcast(I16), ep[:, :, :],
                                SCH_A, SCH_B,
                                mybir.AluOpType.mult, mybir.AluOpType.add,
                            )
                        else:
                            nc.scalar.activation(
                                ex_t[:, :, :], ep[:, :, :],
                                mybir.ActivationFunctionType.Exp,
                            )
                        exq[c] = (ex_t[:, 0, :], ex_t[:, 1, :])
                        if c >= AVLAG:
                            av(c - AVLAG)
                    for c in range(NTB - AVLAG, NTB):
                        av(c)
                    for outp, hl in ((outp1, 0), (outp2, 1)):
                        osb = osbp.tile([XI + 1, RCH], F32)
                        nc.vector.tensor_copy(osb, outp)
                        (nc.scalar if hl == 0 else nc.gpsimd).dma_start(
                            out=outT[hl, :, ts(r, RCH)], in_=osb
                        )
    return nc


def make_in_maps(x, Wq, bq, Wk, bk, Wv, bv):
    X = np.ascontiguousarray(np.asarray(x, dtype=np.float32).reshape(T, D))
    # (NTB, 128, NKT, 128): [j, p, k, t] = X[128j+t, 128k+p] -- every SBUF
    # partition reads one contiguous run per projection slab DMA
    xTm = np.ascontiguousarray(
        X.astype(BF16_NP).reshape(NTB, 128, NKT, 128).transpose(0, 3, 2, 1)
    )
    in_maps = []
    for c in range(NCORE):
        wcols, bcols = [], []
        for h in (2 * c, 2 * c + 1):
            for W, b in ((Wq, bq), (Wk, bk), (Wv, bv)):
                wcols.append(np.asarray(W, np.float32)[h::H, :].T)
                bcols.append(np.asarray(b, np.float32)[h::H])
        w6m = np.ascontiguousarray(
            np.concatenate(wcols, axis=1).astype(BF16_NP)
        )
        b6m = np.ascontiguousarray(
            np.broadcast_to(np.concatenate(bcols), (128, W6)), dtype=np.float32
        )
        in_maps.append({"xT": xTm, "w6": w6m, "b6": b6m})
    return X, in_maps


def assemble(X, results, gamma):
    O = np.empty((T, EG, H), dtype=np.float32)
    for c in range(NCORE):
        res = results[c]
        for hl in range(2):
            h = 2 * c + hl
            onn = res["outT"][hl][0:XI, :]                # (64, 3072)
            s = res["outT"][hl][XI, :]                    # (3072,)
            O[:, :, h] = (onn / s[None, :]).T
    out = O.reshape(T, D)
    g = np.float32(np.asarray(gamma))
    return (g * out + X).reshape(1, 1, T, D).astype(np.float32)


_PROGRAM = None
last_run_info = {}


def kernel(x, Wq, bq, Wk, bk, Wv, bv, gamma):
    global _PROGRAM
    from concourse import bass_utils

    X, in_maps = make_in_maps(x, Wq, bq, Wk, bk, Wv, bv)
    if _PROGRAM is None:
        _PROGRAM = build_program()
        # required for this toolchain's walrus (1 sync wait per instruction);
        # applied here so CoreSim (which predates these NoOps) can still run
        # the unsplit program from build_program()
        _split_multiwaits(_PROGRAM)
    res = bass_utils.run_bass_kernel_spmd(
        _PROGRAM, in_maps, core_ids=list(range(NCORE))
    )
    last_run_info["exec_time_ns"] = res.exec_time_ns
    last_run_info["trace"] = res.instructions_and_trace
    return assemble(X, res.results, gamma)


# revision 24
# speedup vs baseline: 1.2506x; 1.0215x over previous
"""Trainium2 Bass kernel for nn_MultiHeadAttention_68865505624655.

Strategy (head parallelism, 8 cores x 2 heads):
  The reference's reshape(B,-1,T,H) mixes time/channel dims. For head h the
  per-head matrices are exactly reinterpretations of the compacted projection
  output Y_h = X @ W[h::16].T (shape (3072, 64)):
      Q_h^T (xi, t2)  == Y_h viewed as (64, 3072)   (same linear memory!)
      K_h^T (xi, t2)  == same
      V_h  (t2', xi)  == transpose of that view     (needs a real transpose)
  v3 (this file): everything bf16 on the PE (fp32 matmuls run at half rate:
  fp32_mode=HIGH streams 2 cycles/row), exp alternates between TWO engines
  in wide FD=1024 instructions so the PE never stalls on softmax, and every
  DMA stream is spread across otherwise-idle engine queues.
  Each core:
    1. fused QKV projection for its 2 heads: Y6 = X @ [Wq1|Wk1|Wv1|Wq2|Wk2|Wv2]^T
       in bf16 (24 t-blocks x 8 k-tiles), + bias on DVE, written bf16 to DRAM.
    2. reads back Q^T/K^T/V^T as (64,3072) contiguous views (6KB runs/partition),
       four loads on four queues; V^T -> V PE transposes (row-paired) fill the
       PE-idle bubble while those loads stream.
    3. attention, r-chunk (512) outer, c-tile (128) inner, software-pipelined:
         - energy pair S^T[c,r] = K_h^T.T @ Q_h^T for both heads concurrently
           in disjoint PE row groups (bf16, N=512)
         - exp over both heads in ONE FD=1024 instruction, alternating by c
           parity: even c on VectorE via the Schraudolph bit trick
           bf16(exp(x)) ~= bitcast_i16(round(x*128/ln2 + 16248.3)) (a single
           tensor_scalar mult+add); odd c on ScalarE (exact LUT exp).
           Softmax needs no max-subtraction: |S| < ~70 so fp32/bf16 exp
           cannot overflow, and S*A+B stays inside int16.
         - AV (lagged 2 iters): one bf16 matmul per (c,head) with
           lhsT = [V_c | 1] (M=65) accumulates out^T[xi,r] AND the softmax
           denominator Sigma[r] (row 64) over c in PSUM.
    4. writes per-head [out^T; Sigma] (65,3072) tiles per core.
  Host: divide rows 0:64 by row 64, interleave heads into (T,D), gamma*out+x.
  Toolchain workarounds: _split_multiwaits (this walrus allows one sync wait
  per instruction) and _install_ntff_shim (axon NTFF profiling hook).
"""

import sys

if "/opt/trn_rl_repo" not in sys.path:
    sys.path.insert(0, "/opt/trn_rl_repo")

import numpy as np
import ml_dtypes


def _install_ntff_shim():
    """concourse.bass_utils under axon imports antenv.axon_hooks when
    tracing is requested; this image's antenv lacks that submodule.
    Register an equivalent shim (backed by the boot image's ctypes NTFF
    driver) so BASS_TRACE=1 profiles instead of crashing."""
    import types

    if "antenv.axon_hooks" in sys.modules:
        return
    mod = types.ModuleType("antenv.axon_hooks")
    cell = {}

    def get_axon_ntff_profile_hook():
        if "h" not in cell:
            try:
                from trn_agent_boot.trn_boot import _ntff_profile_via_ctypes
                cell["h"] = _ntff_profile_via_ctypes("/opt/axon/libaxon_pjrt.so")
            except Exception:
                cell["h"] = None
        return cell["h"]

    def set_axon_ntff_profile_hook(h):
        cell["h"] = h

    mod.get_axon_ntff_profile_hook = get_axon_ntff_profile_hook
    mod.set_axon_ntff_profile_hook = set_axon_ntff_profile_hook
    sys.modules["antenv.axon_hooks"] = mod


_install_ntff_shim()

import concourse.bass as bass
import concourse.mybir as mybir
import concourse.tile as tile
from concourse.bass import ds, ts
from concourse.masks import make_identity

F32 = mybir.dt.float32
BF16 = mybir.dt.bfloat16
I16 = mybir.dt.int16
BF16_NP = ml_dtypes.bfloat16

T = 3072          # sequence length (and t2 size)
D = 1024          # model dim
H = 16            # heads
NCORE = 8
EG = 64           # channel groups per head (columns of Y_h)
XI = 64           # "feature" dim of the quirky attention (t // 48)
NKT = D // 128    # 8 contraction tiles for the projection
NTB = T // 128    # 24 t-blocks / c-tiles
RCH = 512         # r-chunk (free dim of energy/AV matmuls)
NR = T // RCH     # 6 r-chunks
W6 = 6 * EG       # 384 fused projection output columns

# Schraudolph fast-exp constants for a bf16 result via int16 bit pattern:
# bf16_bits(exp(x)) ~= round(x * 2^7/ln2 + (127*2^7 - c)), c ~= 7.7
SCH_A = 128.0 / float(np.log(2.0))
SCH_B = 16248.3


def _split_multiwaits(nc):
    """This toolchain's walrus accepts at most ONE sync wait per
    instruction (setupSyncWait: 'Too many sync wait commands'), but Tile
    attaches several. Hoist all but the last wait of each instruction onto
    same-engine NoOps inserted right before it — semantically identical
    (sem-ge waits executed in sequence)."""
    n = 0
    for fn in nc.m.functions:
        for bb in fn.blocks:
            insts = list(bb.instructions)
            out = []
            changed = False
            for inst in insts:
                si = inst.sync_info
                if si is not None and len(si.on_wait) > 1:
                    waits = list(si.on_wait)
                    for w in waits[:-1]:
                        n += 1
                        out.append(mybir.InstNoOp(
                            name=f"I-splitwait-{n}",
                            ins=[], outs=[], engine=inst.engine,
                            sync_info=mybir.SyncInfo(on_wait=[w], on_update=[]),
                        ))
                    inst.sync_info = mybir.SyncInfo(
                        on_wait=[waits[-1]], on_update=list(si.on_update)
                    )
                    changed = True
                out.append(inst)
            if changed:
                bb.instructions = out
    return n


def build_program():
    nc = bass.Bass()

    xT = nc.dram_tensor("xT", [NTB, 128, NKT, 128], BF16, kind="ExternalInput")
    w6 = nc.dram_tensor("w6", [D, W6], BF16, kind="ExternalInput")
    b6 = nc.dram_tensor("b6", [128, W6], F32, kind="ExternalInput")
    # y6 is declared in the READ layout: plane q viewed as (xi, t2) -- the
    # (64, 3072) reinterpretation of Y_h -- so the attention-side loads are
    # plain 2D slices with 6KB contiguous runs per partition (one DMA
    # descriptor burst each). The write side pays the scatter instead, where
    # consecutive-partition 128B chunks aggregate into large DRAM bursts.
    y6 = nc.dram_tensor("y6", [6, XI, T], BF16, kind="Internal")
    outT = nc.dram_tensor("outT", [2, XI + 1, T], F32, kind="ExternalOutput")

    with tile.TileContext(nc) as tc:
        with tc.tile_pool(name="const", bufs=1) as constp:
            w6_sb = constp.tile([128, NKT, W6], BF16)
            w6v = w6[:, :].rearrange("(k p) n -> k p n", p=128)
            # startup loads: keep nc.sync free for the first x-tile (critical
            # path to the first matmul); weights go on scalar/gpsimd
            w6_engs = (nc.scalar, nc.gpsimd)
            for k in range(NKT):
                w6_engs[k % 2].dma_start(out=w6_sb[:, k, :], in_=w6v[k, :, :])
            b6_sb = constp.tile([128, W6], F32)
            nc.scalar.dma_start(out=b6_sb, in_=b6[:, :])
            # identity blocks at partitions 0:64 and 64:128 so the two heads'
            # V^T transposes run row-paired in the PE array
            ident = constp.tile([128, 64], BF16)
            nc.gpsimd.memset(ident, 0.0)
            make_identity(nc, ident[0:64, :], nomemset=True)
            make_identity(nc, ident[64:128, :], nomemset=True)
            kt_sb = constp.tile([128, T], BF16)   # rows 0:64 h1 K^T, 64:128 h2
            vt_sb = constp.tile([128, T], BF16)   # rows 0:64 h1 V^T, 64:128 h2
            # V tiles augmented with a ones column: [:, c, h, 0:64] = V_h
            # c-tile, [:, c, h, 64] = 1.0 so one matmul computes out^T AND
            # Sigma (row 64)
            v12_sb = constp.tile([128, NTB, 2, XI + 1], BF16)
            nc.vector.memset(v12_sb[:, :, :, XI:XI + 1], 1.0)

            # ---------------- projection: Y6 = X @ W6^T + b6 ----------------
            with tc.tile_pool(name="xt", bufs=4) as xtp, \
                 tc.tile_pool(name="psy", bufs=4, space="PSUM") as psyp, \
                 tc.tile_pool(name="ysb", bufs=4) as ysbp:
                # y6 write view: plane q, row t = 48*xi + a, col e --
                # exactly the transpose scatter of a 128-row t-block
                y6w = y6.rearrange("q xi (a e) -> (xi a) q e", e=EG)
                for j in range(NTB):
                    xt = xtp.tile([128, NKT, 128], BF16)
                    # merged 1D free dim (2KB/partition contiguous both
                    # sides) and alternating queues: one queue can't sustain
                    # the rate this stream needs to hide under the matmuls
                    (nc.sync if j % 2 == 0 else nc.gpsimd).dma_start(
                        out=xt.rearrange("p k t -> p (k t)"),
                        in_=xT[j, :, :, :].rearrange("p k t -> p (k t)"))
                    psy = psyp.tile([128, W6], F32)
                    for k in range(NKT):
                        nc.tensor.matmul(
                            psy, xt[:, k, :], w6_sb[:, k, :],
                            start=(k == 0), stop=(k == NKT - 1),
                        )
                    ysb = ysbp.tile([128, W6], BF16)
                    nc.vector.tensor_add(ysb, psy, b6_sb)
                    nc.scalar.dma_start(
                        out=y6w[ts(j, 128), :, :],
                        in_=ysb.rearrange("t (q e) -> t q e", q=6),
                    )

            # ------- load K^T / V^T as contiguous (64, 3072) reinterpretations
            # (plain 2D slices of y6 -> 6KB descriptors); V^T first (the PE
            # transposes consume it first). The gpsimd SWDGE is slow for
            # these, so it only gets one; the HWDGE queues take the rest.
            for eng, q, bufap in (
                    (nc.gpsimd, 2, vt_sb[0:64, :]),
                    (nc.sync, 5, vt_sb[64:128, :]),
                    (nc.sync, 1, kt_sb[0:64, :]),
                    (nc.scalar, 4, kt_sb[64:128, :])):
                eng.dma_start(out=bufap, in_=y6[q, :, :])

            # ------- V tiles: true transpose of V^T chunks via the PE, filling
            # the PE-idle bubble while K^T/Q^T stream in. Both heads write one
            # psum tile (disjoint row groups -> concurrent) drained by a
            # single DVE copy, so the scheduler can't unpair them.
            with tc.tile_pool(name="vtps", bufs=2, space="PSUM") as vtpsp:
                for c in range(NTB):
                    # [128, 2, 1024]: each head's transpose drains into its
                    # OWN psum bank (concurrent row-group matmuls must not
                    # share a bank), one strided DVE copy drains both
                    vp = vtpsp.tile([128, 2, 1024], BF16)
                    for hl, row0 in ((0, 0), (1, 64)):
                        nc.tensor.transpose(
                            vp[:, hl, 0:XI], vt_sb[row0:row0 + 64, ts(c, 128)],
                            ident[row0:row0 + 64, :],
                        )
                    nc.vector.tensor_copy(v12_sb[:, c, :, 0:XI], vp[:, :, 0:XI])

            # --------------------------- attention --------------------------
            # Pipeline depths: each head's exp has ~0.7us latency from its
            # energy matmul. Energy psum tiles are ONE BANK each (per head),
            # pool bufs=6, so the energy matmul of (c+3) reuses a slot its
            # exp freed long ago; the AV matmuls of (c) run at c+3 (lag 3).
            # The PE never waits in steady state.
            AVLAG = 3
            with tc.tile_pool(name="qt", bufs=2) as qtp, \
                 tc.tile_pool(name="eps", bufs=3, space="PSUM") as epp, \
                 tc.tile_pool(name="ex", bufs=5) as expool, \
                 tc.tile_pool(name="outp", bufs=1, space="PSUM") as outpp, \
                 tc.tile_pool(name="osb", bufs=4) as osbp:
                for r in range(NR):
                    qt = qtp.tile([128, RCH], BF16)
                    for q, row0 in ((0, 0), (3, 64)):
                        # r=0 on the gpsimd queue, in parallel with the
                        # kt/vt loads on the two HWDGE queues
                        eng = nc.gpsimd if r == 0 else nc.sync
                        eng.dma_start(
                            out=qt[row0:row0 + 64, :],
                            in_=y6[q, :, ds(r * RCH, RCH)],
                        )
                    outp1 = outpp.tile([XI + 1, RCH], F32)
                    outp2 = outpp.tile([XI + 1, RCH], F32)

                    def av(c):
                        for hl, outp in ((0, outp1), (1, outp2)):
                            nc.tensor.matmul(
                                outp[:, :], v12_sb[:, c, hl, :], exq[c][hl],
                                start=(c == 0), stop=(c == NTB - 1),
                            )

                    exq = {}
                    for c in range(NTB):
                        ep0 = epp.tile([128, RCH], F32)
                        ep1 = epp.tile([128, RCH], F32)
                        for ep, row0 in ((ep0, 0), (ep1, 64)):
                            nc.tensor.matmul(
                                ep,
                                kt_sb[row0:row0 + 64, ts(c, 128)],
                                qt[row0:row0 + 64, :],
                                start=True, stop=True,
                            )
                        ex_t = expool.tile([128, 2, RCH], BF16)
                        # exp: head0 always on ScalarE (exact LUT exp), head1
                        # always on VectorE via the Schraudolph bit trick
                        # (int16 view of the bf16 tile). Each engine runs one
                        # ~0.6us FD=512 instruction per ~0.87us iteration.
                        nc.scalar.activation(
                            ex_t[:, 0, :], ep0,
                            mybir.ActivationFunctionType.Exp,
                        )
                        nc.vector.tensor_scalar(
                            ex_t[:, 1, :].bitcast(I16), ep1,
                            SCH_A, SCH_B,
                            mybir.AluOpType.mult, mybir.AluOpType.add,
                        )
                        exq[c] = (ex_t[:, 0, :], ex_t[:, 1, :])
                        if c >= AVLAG:
                            av(c - AVLAG)
                    for c in range(NTB - AVLAG, NTB):
                        av(c)
                    last = r == NR - 1
                    for outp, hl in ((outp1, 0), (outp2, 1)):
                        osb = osbp.tile([XI + 1, RCH], F32)
                        if hl == 0:
                            nc.scalar.copy(osb, outp)
                        else:
                            nc.vector.tensor_copy(osb, outp)
                        if last:
                            # split the final (tail-critical) writes in two
                            # halves on different queues
                            h0 = (XI + 1) // 2
                            engs = ((nc.scalar, nc.sync) if hl == 0
                                    else (nc.gpsimd, nc.sync))
                            engs[0].dma_start(
                                out=outT[hl, 0:h0, ts(r, RCH)],
                                in_=osb[0:h0, :])
                            engs[1].dma_start(
                                out=outT[hl, h0:XI + 1, ts(r, RCH)],
                                in_=osb[h0:XI + 1, :])
                        else:
                            (nc.scalar if hl == 0 else nc.gpsimd).dma_start(
                                out=outT[hl, :, ts(r, RCH)], in_=osb
                            )
    return nc


def make_in_maps(x, Wq, bq, Wk, bk, Wv, bv):
    X = np.ascontiguousarray(np.asarray(x, dtype=np.float32).reshape(T, D))
    # (NTB, 128, NKT, 128): [j, p, k, t] = X[128j+t, 128k+p] -- every SBUF
    # partition reads one contiguous run per projection slab DMA
    xTm = np.ascontiguousarray(
        X.astype(BF16_NP).reshape(NTB, 128, NKT, 128).transpose(0, 3, 2, 1)
    )
    in_maps = []
    for c in range(NCORE):
        wcols, bcols = [], []
        for h in (2 * c, 2 * c + 1):
            for W, b in ((Wq, bq), (Wk, bk), (Wv, bv)):
                wcols.append(np.asarray(W, np.float32)[h::H, :].T)
                bcols.append(np.asarray(b, np.float32)[h::H])
        w6m = np.ascontiguousarray(
            np.concatenate(wcols, axis=1).astype(BF16_NP)
        )
        b6m = np.ascontiguousarray(
            np.broadcast_to(np.concatenate(bcols), (128, W6)), dtype=np.float32
        )
        in_maps.append({"xT": xTm, "w6": w6m, "b6": b6m})
    return X, in_maps


def assemble(X, results, gamma):
    O = np.empty((T, EG, H), dtype=np.float32)
    for c in range(NCORE):
        res = results[c]
        for hl in range(2):
            h = 2 * c + hl
            onn = res["outT"][hl][0:XI, :]                # (64, 3072)
            s = res["outT"][hl][XI, :]                    # (3072,)
            O[:, :, h] = (onn / s[None, :]).T
    out = O.reshape(T, D)
    g = np.float32(np.asarray(gamma))
    return (g * out + X).reshape(1, 1, T, D).astype(np.float32)


_PROGRAM = None
last_run_info = {}


def kernel(x, Wq, bq, Wk, bk, Wv, bv, gamma):
    global _PROGRAM
    from concourse import bass_utils

    X, in_maps = make_in_maps(x, Wq, bq, Wk, bk, Wv, bv)
    if _PROGRAM is None:
        _PROGRAM = build_program()
        # required for this toolchain's walrus (1 sync wait per instruction);
        # applied here so CoreSim (which predates these NoOps) can still run
        # the unsplit program from build_program()
        _split_multiwaits(_PROGRAM)
    res = bass_utils.run_bass_kernel_spmd(
        _PROGRAM, in_maps, core_ids=list(range(NCORE))
    )
    last_run_info["exec_time_ns"] = res.exec_time_ns
    last_run_info["trace"] = res.instructions_and_trace
    return assemble(X, res.results, gamma)
